# revision 41
# baseline (speedup 1.0000x reference)
"""Trainium2 Bass kernel for DEMONet-style GNN message passing (2 layers + pool).

Strategy: shard the 50000 nodes across 8 NeuronCores (degree-balanced deal),
each core owning its nodes' outgoing edges.  Neighbor MEAN is computed on the
TensorEngine in "flipped" orientation: per 128-edge tile,
  mT_psum[f, src] += G_tile[e, f]^T-contract @ S_tile[e, src]
where G_tile is a [128-edge, D] tile fetched with the GPSIMD dma_gather
extended instruction (int16 indices -> the node table is split in two
<32768-row halves) and S_tile is an edge->src-slot one-hot PRE-SCALED by
1/deg(src) (built in a single DVE tensor_scalar: is_equal then mult with two
per-partition f32 scalar pointers).  The aggregated mean lands transposed
[feat, src], which feeds the Wl transform matmul directly as lhsT --- no
PE transposes anywhere.  z = x@(Wg+Ws) + mean@Wl accumulates in one PSUM
group per block; elu is Exp/Relu on ACT plus two cheap bf16 DVE ops.
Layer 1 runs from a replicated h1 table assembled on the host between the two
launches; the graph-level mean pool is reduced on-chip to a [64, 256] partial
per core and finished on the host (tiny classifier matmul).
"""
import numpy as np
import ml_dtypes

import concourse.bass as bass
import concourse.bacc as bacc
import concourse.tile as tile
from concourse import mybir
from concourse.bass_utils import run_bass_kernel_spmd

# ---------------------------------------------------------------- constants
N_NODES = 50000
N_EDGES = 800000
IN_DIM = 128
HIDDEN = 256
N_CLASSES = 10
N_GRAPHS = 64
N_CORES = 8
HALF = 32768                      # int16 index limit -> split tables
NPC = N_NODES // N_CORES          # 6250 nodes per core
NBLK = 49                         # ceil(6250/128)
SLOTS = NBLK * 128                # 6272 padded slots
CW = 8                            # tiles per dma_gather call (1024 idx)
SW = 8                            # S tiles per DVE build buffer
F32 = mybir.dt.float32
BF16 = mybir.dt.bfloat16
I16 = mybir.dt.int16
FP8 = mybir.dt.float8e4

_CACHE = {}


# ------------------------------------------------------------ host helpers
def _pack_idxs(flat):
    """flat int array (len % 128 == 0) -> [128, len//16] int16, wrapped in 16
    partitions and replicated 8x down the partition dim (dma_gather layout)."""
    n = len(flat)
    w = np.zeros((16, n // 16), np.int16)
    w[np.arange(n) % 16, np.arange(n) // 16] = flat
    return np.ascontiguousarray(np.tile(w, (8, 1)))


def _elu(z):
    return np.where(z > 0, z, np.expm1(np.minimum(z, 0.0))).astype(np.float32)


def _preprocess(edge_index, batch):
    src = np.asarray(edge_index[0], dtype=np.int64)
    dst = np.asarray(edge_index[1], dtype=np.int64)
    batch = np.asarray(batch, dtype=np.int64)

    deg = np.bincount(src, minlength=N_NODES).astype(np.float32)

    order = np.argsort(-deg, kind="stable")          # rank -> node id
    perm = [order[c::N_CORES] for c in range(N_CORES)]   # per-core node ids
    core_of = np.empty(N_NODES, np.int64)
    slot_of = np.empty(N_NODES, np.int64)
    # degree-balanced: i-th (degree-ranked) node of a core -> block i % NBLK,
    # row i // NBLK, so every 128-slot block sees the same degree mix.
    slot_arr = (np.arange(NPC) % NBLK) * 128 + np.arange(NPC) // NBLK
    for c in range(N_CORES):
        core_of[perm[c]] = c
        slot_of[perm[c]] = slot_arr

    ecore = core_of[src]
    eslot = slot_of[src]
    eblk = eslot // 128
    esrc = eslot % 128
    ehalf = (dst >= HALF).astype(np.int64)

    # edges per (core, block, half)
    grp = (ecore * NBLK + eblk) * 2 + ehalf
    cnt = np.bincount(grp, minlength=N_CORES * NBLK * 2).reshape(N_CORES, NBLK, 2)
    ntile_per = -(-cnt // 128)                        # ceil
    NT0 = ntile_per[:, :, 0].max(axis=0)              # per-block, max over cores
    NT1 = ntile_per[:, :, 1].max(axis=0)
    NT0 = np.maximum(NT0, 1)                          # keep PSUM group non-empty

    # global tile order: all half-0 tiles (block-major), then all half-1 tiles.
    tile_base = np.zeros((NBLK, 2), np.int64)         # first tile id of (b, h)
    t = 0
    for b in range(NBLK):
        tile_base[b, 0] = t
        t += int(NT0[b])
    TOT0 = t
    for b in range(NBLK):
        tile_base[b, 1] = t
        t += int(NT1[b])
    SUMNT = t
    TOT1 = SUMNT - TOT0
    NIDX = SUMNT * 128

    # absolute edge positions
    base_flat = np.zeros(N_CORES * NBLK * 2, np.int64)
    for b in range(NBLK):
        for h in (0, 1):
            base_flat[np.arange(N_CORES) * NBLK * 2 + b * 2 + h] = tile_base[b, h] * 128
    ordr = np.argsort(grp, kind="stable")
    gs = grp[ordr]
    starts = np.r_[0, np.flatnonzero(np.diff(gs)) + 1]
    seg_len = np.diff(np.r_[starts, len(gs)])
    ccount = np.arange(len(gs)) - np.repeat(starts, seg_len)
    pos = np.empty(N_EDGES, np.int64)
    pos[ordr] = ccount
    abspos = base_flat[grp] + pos

    dinv = 1.0 / np.maximum(deg, 1.0)

    idx_flat = np.zeros((N_CORES, NIDX), np.int64)
    src_flat = np.full((N_CORES, NIDX), -1.0, np.float32)
    dinv_flat = np.ones((N_CORES, NIDX), np.float32)
    idx_flat[ecore, abspos] = dst - HALF * ehalf
    src_flat[ecore, abspos] = esrc
    dinv_flat[ecore, abspos] = dinv[src]

    idx_packed = [_pack_idxs(idx_flat[c]) for c in range(N_CORES)]
    srcf = [np.ascontiguousarray(src_flat[c].reshape(SUMNT, 128).T) for c in range(N_CORES)]
    dinvf = [np.ascontiguousarray(dinv_flat[c].reshape(SUMNT, 128).T) for c in range(N_CORES)]

    batchf = []
    for c in range(N_CORES):
        arr = np.zeros(SLOTS, np.float32)
        arr[slot_arr] = batch[perm[c]].astype(np.float32)
        # [128, NBLK]: column b, partition p = graph id of slot (b, p)
        batchf.append(np.ascontiguousarray(arr.reshape(NBLK, 128).T))

    colidx = np.ascontiguousarray(
        np.tile(np.arange(128, dtype=np.float32)[None, :], (128, 1))
    ).astype(ml_dtypes.bfloat16)

    # -------- layer-0 "leftover" layout: full tiles at floor(min-core/128)
    # (zero pads), remainders packed densely into shared tiles whose edges may
    # span several blocks; those are aggregated in a preload pass.
    F = (cnt.min(axis=0) // 128).astype(np.int64)            # [NBLK, 2]
    NFULL0, NFULL1 = int(F[:, 0].sum()), int(F[:, 1].sum())
    fullbase = np.zeros((NBLK, 2), np.int64)
    fullbase[1:, 0] = np.cumsum(F[:, 0])[:-1]
    fullbase[1:, 1] = np.cumsum(F[:, 1])[:-1]
    l = cnt - 128 * F[None, :, :]                            # [C, NBLK, 2] >= 0
    loff = np.zeros_like(l)
    loff[:, 1:, :] = np.cumsum(l, axis=1)[:, :-1, :]
    LT = [int(-(-l[:, :, h].sum(axis=1).max() // 128)) for h in (0, 1)]
    assert LT[0] >= 1
    T_h = [LT[0] + NFULL0, LT[1] + NFULL1]
    LSUM = T_h[0] + T_h[1]
    LNIDX = LSUM * 128

    eF = F[eblk, ehalf]
    isfull = pos < 128 * eF
    lpos = loff[ecore, eblk, ehalf] + (pos - 128 * eF)
    LTarr = np.array(LT, np.int64)
    tile_local = np.where(isfull, LTarr[ehalf] + fullbase[eblk, ehalf] + pos // 128,
                          lpos // 128)
    row = np.where(isfull, pos % 128, lpos % 128)
    halfbase = np.where(ehalf == 1, T_h[0], 0)
    abspos_lo = (halfbase + tile_local) * 128 + row

    # pair list: (h, leftover-tile, block) present on ANY core. Each half gets
    # its own accumulators/preload (so quad PSUM windows stay monotone), so
    # every block needs at least one pair IN EACH HALF (dummy = all-masked).
    lmask = ~isfull
    pk = ehalf[lmask] * 1000000 + (lpos[lmask] // 128) * 1000 + eblk[lmask]
    upair = np.unique(pk)
    assert LT[1] >= 1
    pair_list = []
    for h in (0, 1):
        ks = upair[(upair // 1000000) == h]
        seen = set((ks % 1000).tolist())
        pair_list += [(h, 0, b) for b in range(NBLK) if b not in seen]
        pair_list += [(h, int((k // 1000) % 1000), int(k % 1000)) for k in ks]
    NP = len(pair_list)
    pair_col = {}
    for j, (h, t, b) in enumerate(pair_list):
        pair_col[(h, t, b)] = j
    first_of, last_of, quad_last = {}, {}, {}
    for j, (h, t, b) in enumerate(pair_list):
        first_of.setdefault((h, b), j)
        last_of[(h, b)] = j
        quad_last[(h, b // 4)] = j
    pair_sched = [(h, t, b, j, j == first_of[(h, b)], j == last_of[(h, b)],
                   quad_last[(h, b // 4)] == j)
                  for j, (h, t, b) in enumerate(pair_list)]

    # srcf/dinvf columns: [0, NP) pairs, [NP, NP+NFULL) full tiles (h0 then h1)
    NFULL = NFULL0 + NFULL1
    idx_lo = np.zeros((N_CORES, LNIDX), np.int64)
    src_lo = np.full((N_CORES, 128, NP + NFULL), -1.0, np.float32)
    dinv_lo = np.ones((N_CORES, 128, NP + NFULL), np.float32)
    idx_lo[ecore, abspos_lo] = dst - HALF * ehalf
    fullcol = NP + np.where(ehalf == 1, NFULL0, 0) + fullbase[eblk, ehalf] + pos // 128
    src_lo[ecore[isfull], row[isfull], fullcol[isfull]] = esrc[isfull]
    dinv_lo[ecore[isfull], row[isfull], fullcol[isfull]] = dinv[src[isfull]]
    pcols = np.array([pair_col[(int(h), int(t), int(b))] for h, t, b in
                      zip(ehalf[lmask], lpos[lmask] // 128, eblk[lmask])], np.int64)
    src_lo[ecore[lmask], row[lmask], pcols] = esrc[lmask]
    dinv_lo[ecore[lmask], row[lmask], pcols] = dinv[src[lmask]]

    idx_packed_lo = [_pack_idxs(idx_lo[c]) for c in range(N_CORES)]
    srcf_lo = [np.ascontiguousarray(src_lo[c]) for c in range(N_CORES)]
    dinvf_lo = [np.ascontiguousarray(dinv_lo[c]) for c in range(N_CORES)]
    ident = np.eye(128, dtype=ml_dtypes.bfloat16)

    return dict(deg=deg, perm=perm, slot_arr=slot_arr, NT0=NT0, NT1=NT1,
                TOT0=TOT0, TOT1=TOT1,
                tile_base=tile_base, SUMNT=SUMNT, NIDX=NIDX,
                idx_packed=idx_packed, srcf=srcf, dinvf=dinvf, batchf=batchf,
                colidx=colidx, batch=batch,
                F=F, fullbase=fullbase, LT=LT, T_h=T_h, LSUM=LSUM, LNIDX=LNIDX,
                NP=NP, NFULL0=NFULL0, NFULL1=NFULL1, pair_sched=pair_sched,
                idx_packed_lo=idx_packed_lo, srcf_lo=srcf_lo, dinvf_lo=dinvf_lo,
                ident=ident)


# ------------------------------------------------------------ device program
def _build_program(layer, pre):
    """layer 0: x -> h1 staging.  layer 1: h1 -> pooled partial [64, 256]."""
    D = IN_DIM if layer == 0 else HIDDEN      # raw feature width (gather table)
    NDC = D // 128                            # 128-row chunks of D
    LOMODE = (layer == 0)                     # leftover-packed tile layout
    tile_base = pre["tile_base"]
    NT0, NT1 = pre["NT0"], pre["NT1"]
    TOT0, TOT1 = pre["TOT0"], pre["TOT1"]
    if LOMODE:
        NP = pre["NP"]
        SUMNT = NP + pre["NFULL0"] + pre["NFULL1"]     # srcf/dinvf columns
        NIDX = pre["LNIDX"]
        LTh = pre["LT"]
        htot_v = pre["T_h"]
        scolbase = [NP - LTh[0], NP + pre["NFULL0"] - LTh[1]]
        Fb, fullbase = pre["F"], pre["fullbase"]
    else:
        SUMNT, NIDX = pre["SUMNT"], pre["NIDX"]
        LTh = [0, 0]
        htot_v = [TOT0, TOT1]
        scolbase = [0, TOT0]

    nc = bacc.Bacc(dynamic_dma_scratch_size=65536)
    tab = nc.declare_dram_parameter("tab", [N_NODES, D], BF16, isOutput=False)
    hT = nc.declare_dram_parameter("hT", [D, SLOTS], FP8, isOutput=False)
    Wgs = nc.declare_dram_parameter("Wgs", [D, HIDDEN], BF16, isOutput=False)
    Wl = nc.declare_dram_parameter("Wl", [D, HIDDEN], BF16, isOutput=False)
    idxs = nc.declare_dram_parameter("idxs", [128, NIDX // 16], I16, isOutput=False)
    srcf = nc.declare_dram_parameter("srcf", [128, SUMNT], F32, isOutput=False)
    dinvf = nc.declare_dram_parameter("dinvf", [128, SUMNT], F32, isOutput=False)
    colidx = nc.declare_dram_parameter("colidx", [128, 128], BF16, isOutput=False)
    if layer == 0:
        h1st = nc.declare_dram_parameter("h1st", [128, NBLK * HIDDEN], FP8, isOutput=True)
        identp = nc.declare_dram_parameter("identp", [128, 128], BF16, isOutput=False)
    else:
        batchf = nc.declare_dram_parameter("batchf", [128, NBLK], F32, isOutput=False)
        pool_out = nc.declare_dram_parameter("pool_out", [N_GRAPHS, HIDDEN], F32, isOutput=True)

    import contextlib
    with tile.TileContext(nc) as tc:
        with (
            tc.tile_pool(name="const", bufs=1) as cpool,
            tc.tile_pool(name="gbuf", bufs=10) as gpool,
            tc.tile_pool(name="sbuf4", bufs=6) as spool,
            tc.tile_pool(name="mt", bufs=3) as mpool,
            tc.tile_pool(name="elu", bufs=3) as epool,
            tc.tile_pool(name="psum", bufs=2, space="PSUM") as pp,
            tc.tile_pool(name="psacc", bufs=1, space="PSUM") as pacc,
            contextlib.ExitStack() as _stk,
        ):
            lfpool = (_stk.enter_context(tc.tile_pool(name="lf", bufs=3, space="PSUM"))
                      if LOMODE else None)
            idxs_sb = cpool.tile([128, NIDX // 16], I16)
            nc.sync.dma_start(out=idxs_sb[:], in_=idxs[:])
            srcf_sb = cpool.tile([128, SUMNT], F32)
            nc.sync.dma_start(out=srcf_sb[:], in_=srcf[:])
            dinvf_sb = cpool.tile([128, SUMNT], F32)
            nc.sync.dma_start(out=dinvf_sb[:], in_=dinvf[:])
            colidx_sb = cpool.tile([128, 128], BF16)
            nc.sync.dma_start(out=colidx_sb[:], in_=colidx[:])
            hT_sb, Wgs_sb, Wl_sb = [], [], []
            for dci in range(NDC):
                rows = slice(dci * 128, (dci + 1) * 128)
                th = cpool.tile([128, SLOTS], FP8, tag=f"hT{dci}")
                nc.sync.dma_start(out=th[:], in_=hT[rows, :])
                hT_sb.append(th)
                tg = cpool.tile([128, HIDDEN], BF16, tag=f"Wgs{dci}")
                nc.sync.dma_start(out=tg[:], in_=Wgs[rows, :])
                Wgs_sb.append(tg)
                tl = cpool.tile([128, HIDDEN], BF16, tag=f"Wl{dci}")
                nc.sync.dma_start(out=tl[:], in_=Wl[rows, :])
                Wl_sb.append(tl)
            if layer == 0:
                stage = cpool.tile([128, NBLK * HIDDEN], FP8)
            if layer == 1:
                batchf_sb = cpool.tile([128, NBLK], F32)
                nc.sync.dma_start(out=batchf_sb[:], in_=batchf[:])
                Bpool_sb = cpool.tile([128, NBLK * N_GRAPHS], BF16)
                for b in range(NBLK):
                    # Bpool[p, b*64+j] = (batch(slot b,p) == j), built on DVE
                    nc.vector.tensor_scalar(
                        out=Bpool_sb[:, b * N_GRAPHS:(b + 1) * N_GRAPHS],
                        in0=colidx_sb[:, :N_GRAPHS],
                        scalar1=batchf_sb[:, b:b + 1], scalar2=None,
                        op0=mybir.AluOpType.is_equal)
                pool_ps = pacc.tile([N_GRAPHS, HIDDEN], F32, space="PSUM")

            # gather-call buffers and S-tile groups, issued on demand
            gtiles = [[], []]
            sgroups = [[], []]
            ncalls = [0, 0]
            nsg = [0, 0]
            hstart = [0, htot_v[0]]
            fullcnt = [htot_v[0] - LTh[0], htot_v[1] - LTh[1]]

            def need(h, upto_local):
                while ncalls[h] * CW < min(upto_local, htot_v[h]):
                    j = ncalls[h]
                    nt = min(CW, htot_v[h] - j * CW)
                    gb = gpool.tile([128, CW * D], BF16, tag=f"g{h}", name=f"g{h}_{j}")
                    t0 = hstart[h] + j * CW
                    tab_ap = tab[:HALF, :] if h == 0 else tab[HALF:, :]
                    nc.gpsimd.dma_gather(
                        out_ap=gb[:, :nt * D].rearrange("p (t d) -> p t d", t=nt),
                        in_ap=tab_ap,
                        idxs_ap=idxs_sb[:, t0 * 8:(t0 + nt) * 8],
                        num_idxs=nt * 128, num_idxs_reg=nt * 128, elem_size=D,
                    )
                    gtiles[h].append(gb)
                    ncalls[h] += 1
                upto_full = upto_local - LTh[h]
                while nsg[h] * SW < min(upto_full, fullcnt[h]):
                    j = nsg[h]
                    k = min(SW, fullcnt[h] - j * SW)
                    sg = spool.tile([128, SW * 128], BF16, tag=f"S{h}", name=f"S{h}_{j}")
                    for i in range(k):
                        # S[e, src] = dinv[src] * (slot(e) == src): one DVE op
                        col = scolbase[h] + LTh[h] + j * SW + i
                        nc.vector.tensor_scalar(
                            out=sg[:, i * 128:(i + 1) * 128],
                            in0=colidx_sb[:],
                            scalar1=srcf_sb[:, col:col + 1],
                            scalar2=dinvf_sb[:, col:col + 1],
                            op0=mybir.AluOpType.is_equal,
                            op1=mybir.AluOpType.mult)
                    sgroups[h].append(sg)
                    nsg[h] += 1

            # leftover preload pass (layer 0): aggregate the densely packed
            # leftover tiles into quad-width PSUM accumulators, evacuate to an
            # SBUF preload table, which each block's mT group starts from.
            if LOMODE:
                ident_sb = cpool.tile([128, 128], BF16)
                nc.sync.dma_start(out=ident_sb[:], in_=identp[:])
                pre_sb = [cpool.tile([128, NBLK * 128], BF16, tag=f"pre{h}",
                                     name=f"pre{h}") for h in (0, 1)]
                lfps = {}
                for (h, t, w, j, st_, sp_, qlast) in pre["pair_sched"]:
                    need(h, t + 1)
                    gb = gtiles[h][t // CW]
                    gcol = t % CW
                    spt = mpool.tile([128, 128], BF16, tag="P", name=f"P_{j}")
                    nc.vector.tensor_scalar(
                        out=spt[:], in0=colidx_sb[:],
                        scalar1=srcf_sb[:, j:j + 1], scalar2=dinvf_sb[:, j:j + 1],
                        op0=mybir.AluOpType.is_equal, op1=mybir.AluOpType.mult)
                    q = w // 4
                    qw = min(4, NBLK - 4 * q)
                    if (h, q) not in lfps:
                        lfps[(h, q)] = lfpool.tile([128, qw * 128], F32, space="PSUM",
                                                   tag="lf", name=f"lf_{h}_{q}")
                    nc.tensor.matmul(
                        out=lfps[(h, q)][:, (w % 4) * 128:(w % 4 + 1) * 128],
                        lhsT=gb[:, gcol * D:gcol * D + 128], rhs=spt[:],
                        start=st_, stop=sp_, skip_group_check=True)
                    if qlast:
                        nc.scalar.activation(
                            out=pre_sb[h][:, 4 * q * 128:(4 * q + qw) * 128],
                            in_=lfps[(h, q)][:],
                            func=mybir.ActivationFunctionType.Copy)

            # software pipeline across blocks: the Wl matmuls (which wait on
            # the ACT evacuation of mT) lag one block, and the pool matmul
            # (which waits on the full elu chain) lags two, so the Tensor
            # engine never stalls on ACT/DVE results of the current block.
            mT_sb_d, mT_ps_d, z_ps_d, h_d = {}, {}, {}, {}

            def stage1(b):
                if LOMODE:
                    p0 = LTh[0] + int(fullbase[b, 0])
                    n0 = int(Fb[b, 0])
                    p1 = LTh[1] + int(fullbase[b, 1])
                    n1 = int(Fb[b, 1])
                else:
                    p0 = int(tile_base[b, 0])
                    n0 = int(NT0[b])
                    p1 = int(tile_base[b, 1]) - TOT0
                    n1 = int(NT1[b])
                need(0, p0 + n0)
                need(1, p1 + n1)
                tlist = [(0, p0 + i) for i in range(n0)]
                tlist += [(1, p1 + i) for i in range(n1)]

                # flipped mean-aggregation: mT[f, src] accumulated per d-chunk
                mT_ps = [pp.tile([128, 128], F32, space="PSUM", tag=f"mT{f}",
                                 name=f"mT{f}_{b}") for f in range(NDC)]
                if LOMODE:
                    # inject the two half preloads; first opens the PSUM group
                    nc.tensor.matmul(out=mT_ps[0][:], lhsT=ident_sb[:],
                                     rhs=pre_sb[0][:, b * 128:(b + 1) * 128],
                                     start=True, stop=False,
                                     skip_group_check=True)
                    nc.tensor.matmul(out=mT_ps[0][:], lhsT=ident_sb[:],
                                     rhs=pre_sb[1][:, b * 128:(b + 1) * 128],
                                     start=False, stop=(len(tlist) == 0),
                                     skip_group_check=True)
                for k, (h, lt) in enumerate(tlist):
                    gb = gtiles[h][lt // CW]
                    gcol = lt % CW
                    fidx = lt - LTh[h]
                    sg = sgroups[h][fidx // SW]
                    scol = fidx % SW
                    for f in range(NDC):
                        nc.tensor.matmul(
                            out=mT_ps[f][:],
                            lhsT=gb[:, gcol * D + f * 128:gcol * D + (f + 1) * 128],
                            rhs=sg[:, scol * 128:(scol + 1) * 128],
                            start=(k == 0 and not LOMODE),
                            stop=(k == len(tlist) - 1),
                            skip_group_check=True)
                # z global branch (independent of mT)
                z_ps = pp.tile([128, HIDDEN], F32, space="PSUM", tag="z", name=f"z_{b}")
                cols = slice(b * 128, (b + 1) * 128)
                for d in range(NDC):
                    nc.tensor.matmul(out=z_ps[:], lhsT=hT_sb[d][:, cols], rhs=Wgs_sb[d][:],
                                     start=(d == 0), stop=False, skip_group_check=True)
                z_ps_d[b] = z_ps
                mT_ps_d[b] = mT_ps

            def stage2(b):
                # evacuate mT on ACT (bf16) -- emitted after stage3(b-1) so the
                # previous block's exp/relu are not stuck behind this wait
                mT_sb_d[b] = []
                for f in range(NDC):
                    mt = mpool.tile([128, 128], BF16, tag=f"mts{f}", name=f"mts{f}_{b}")
                    nc.scalar.activation(out=mt[:], in_=mT_ps_d[b][f][:],
                                         func=mybir.ActivationFunctionType.Copy)
                    mT_sb_d[b].append(mt)

            def stage3(b):
                z_ps = z_ps_d[b]
                for f in range(NDC):
                    nc.tensor.matmul(out=z_ps[:], lhsT=mT_sb_d[b][f][:], rhs=Wl_sb[f][:],
                                     start=False, stop=(f == NDC - 1), skip_group_check=True)
                # elu(z) = relu(z) + min(exp(z), 1) - 1   (biases are zero)
                e = epool.tile([128, HIDDEN], BF16, tag="e", name=f"e_{b}")
                nc.scalar.activation(out=e[:], in_=z_ps[:],
                                     func=mybir.ActivationFunctionType.Exp)
                r = epool.tile([128, HIDDEN], BF16, tag="r", name=f"r_{b}")
                nc.scalar.activation(out=r[:], in_=z_ps[:],
                                     func=mybir.ActivationFunctionType.Relu)
                u = epool.tile([128, HIDDEN], BF16, tag="u", name=f"u_{b}")
                nc.vector.tensor_scalar(out=u[:], in0=e[:], scalar1=1.0, scalar2=-1.0,
                                        op0=mybir.AluOpType.min, op1=mybir.AluOpType.add)
                if layer == 0:
                    nc.vector.tensor_tensor(out=stage[:, b * HIDDEN:(b + 1) * HIDDEN],
                                            in0=r[:], in1=u[:], op=mybir.AluOpType.add)
                    if b % 5 == 4 or b == NBLK - 1:
                        lo = (b // 5) * 5
                        nc.sync.dma_start(out=h1st[:, lo * HIDDEN:(b + 1) * HIDDEN],
                                          in_=stage[:, lo * HIDDEN:(b + 1) * HIDDEN])
                else:
                    h_sb = epool.tile([128, HIDDEN], BF16, tag="h", name=f"h_{b}")
                    nc.vector.tensor_tensor(out=h_sb[:], in0=r[:], in1=u[:],
                                            op=mybir.AluOpType.add)
                    h_d[b] = h_sb

            def stage5(b):
                nc.tensor.matmul(out=pool_ps[:],
                                 lhsT=Bpool_sb[:, b * N_GRAPHS:(b + 1) * N_GRAPHS],
                                 rhs=h_d.pop(b)[:], start=(b == 0), stop=(b == NBLK - 1),
                                 skip_group_check=True)

            for b in range(NBLK):
                stage1(b)
                if b >= 1:
                    stage3(b - 1)
                stage2(b)
                if layer == 1 and b >= 2:
                    stage5(b - 2)
            stage3(NBLK - 1)
            if layer == 1:
                stage5(NBLK - 2)
                stage5(NBLK - 1)

            if layer == 1:
                po = cpool.tile([N_GRAPHS, HIDDEN], F32)
                nc.scalar.activation(out=po[:], in_=pool_ps[:],
                                     func=mybir.ActivationFunctionType.Copy)
                nc.sync.dma_start(out=pool_out[:], in_=po[:])

    nc.compile()
    return nc


# Legalize for this walrus build: max ONE sync wait per instruction. Split
# extras onto same-engine NoOps just before the over-subscribed instruction.
def _legalize_bir(raw):
    import orjson
    bir = orjson.loads(raw)
    ctr = 0
    for func in bir.get("functions", []):
        for blk in func.get("blocks", []):
            insts = blk.get("instructions") or []
            out = []
            for inst in insts:
                si = inst.get("sync_info")
                waits = (si.get("on_wait") or []) if si else []
                if len(waits) > 1:
                    for w in waits[:-1]:
                        ctr += 1
                        out.append({"debug": inst.get("debug", 0), "engine": inst["engine"],
                                    "ins": [], "outs": [], "name": f"wsplit-{ctr}",
                                    "opcode": "NoOp",
                                    "sync_info": {"on_update": [], "on_wait": [w]}})
                    si["on_wait"] = waits[-1:]
                out.append(inst)
            blk["instructions"] = out
    return orjson.dumps(bir)


_orig_to_json_bytes = bass.Bass.to_json_bytes
if not getattr(bass.Bass, "_wait_legalized", False):
    bass.Bass.to_json_bytes = lambda self: _legalize_bir(_orig_to_json_bytes(self))
    bass.Bass._wait_legalized = True


def _run_with_retry(nc, in_maps, cores, tries=4):
    import time as _time
    last = None
    for att in range(tries):
        try:
            return run_bass_kernel_spmd(nc, in_maps, cores)
        except Exception as e:          # first exec of a fresh NEFF can wedge
            last = e
            _time.sleep(3.0)
    raise last


# ------------------------------------------------------------------- kernel
def kernel(x, edge_index, batch, Wg0, Wl0, Ws0, b0, Wg1, Wl1, Ws1, b1, Wc, bc,
           _profile=False):
    x = np.asarray(x, np.float32)
    Wg0, Wl0, Ws0 = (np.asarray(a, np.float32) for a in (Wg0, Wl0, Ws0))
    Wg1, Wl1, Ws1 = (np.asarray(a, np.float32) for a in (Wg1, Wl1, Ws1))
    b0, b1 = np.asarray(b0, np.float32), np.asarray(b1, np.float32)
    Wc, bc = np.asarray(Wc, np.float32), np.asarray(bc, np.float32)

    pre = _preprocess(edge_index, batch)
    key = pre["SUMNT"]
    if ("p0", key) not in _CACHE:
        _CACHE[("p0", key)] = _build_program(0, pre)
        _CACHE[("p1", key)] = _build_program(1, pre)
    nc0, nc1 = _CACHE[("p0", key)], _CACHE[("p1", key)]

    perm, deg, batch_np = pre["perm"], pre["deg"], pre["batch"]
    cores = list(range(N_CORES))

    # ------------------------------------------------ launch A: layer 0
    Wgs0 = Wg0 + Ws0
    x_bf = x.astype(ml_dtypes.bfloat16)
    Wl0_bf = Wl0.astype(ml_dtypes.bfloat16)
    Wgs0_bf = Wgs0.astype(ml_dtypes.bfloat16)
    in_maps = []
    for c in cores:
        xT = np.zeros((IN_DIM, SLOTS), ml_dtypes.float8_e4m3)
        xT[:, pre["slot_arr"]] = x[perm[c]].T.astype(ml_dtypes.float8_e4m3)
        in_maps.append({
            "tab": x_bf, "hT": xT, "Wgs": Wgs0_bf, "Wl": Wl0_bf,
            "idxs": pre["idx_packed_lo"][c], "srcf": pre["srcf_lo"][c],
            "dinvf": pre["dinvf_lo"][c], "colidx": pre["colidx"],
            "identp": pre["ident"],
        })
    # first 8-core execution of a fresh NEFF can wedge an engine while the
    # GPSIMD library loads race; a 1-core warmup run makes it reliable.
    if ("w0", key) not in _CACHE:
        _run_with_retry(nc0, [in_maps[0]], [0])
        _CACHE[("w0", key)] = True
    resA = _run_with_retry(nc0, in_maps, cores)

    h1 = np.empty((N_NODES, HIDDEN), np.float32)
    for c in cores:
        st = resA.results[c]["h1st"].astype(np.float32).reshape(128, NBLK, HIDDEN)
        h1[perm[c]] = st.transpose(1, 0, 2).reshape(SLOTS, HIDDEN)[pre["slot_arr"]]
    assert not np.any(b0) and not np.any(b1), "kernel assumes zero layer biases"
    deg0 = np.flatnonzero(deg == 0)
    if len(deg0):
        h1[deg0] = _elu(x[deg0] @ Wg0 + b0)

    # ------------------------------------------------ launch B: layer 1
    Wgs1 = Wg1 + Ws1
    h1_bf = h1.astype(ml_dtypes.bfloat16)
    Wl1_bf = Wl1.astype(ml_dtypes.bfloat16)
    Wgs1_bf = Wgs1.astype(ml_dtypes.bfloat16)
    in_maps = []
    for c in cores:
        hT = np.zeros((HIDDEN, SLOTS), ml_dtypes.float8_e4m3)
        hT[:, pre["slot_arr"]] = h1[perm[c]].T.astype(ml_dtypes.float8_e4m3)
        in_maps.append({
            "tab": h1_bf, "hT": hT, "Wgs": Wgs1_bf, "Wl": Wl1_bf,
            "idxs": pre["idx_packed"][c], "srcf": pre["srcf"][c],
            "dinvf": pre["dinvf"][c], "colidx": pre["colidx"],
            "batchf": pre["batchf"][c],
        })
    if ("w1", key) not in _CACHE:
        _run_with_retry(nc1, [in_maps[0]], [0])
        _CACHE[("w1", key)] = True
    resB = _run_with_retry(nc1, in_maps, cores)

    pool_sum = np.zeros((N_GRAPHS, HIDDEN), np.float32)
    for c in cores:
        pool_sum += resB.results[c]["pool_out"]
    if len(deg0):
        h2w = _elu(h1[deg0] @ Wgs1 + b1)
        h2c = _elu(h1[deg0] @ Wg1 + b1)
        np.add.at(pool_sum, batch_np[deg0], h2c - h2w)

    cnt = np.bincount(batch_np, minlength=N_GRAPHS).astype(np.float32)
    g = pool_sum / np.maximum(cnt, 1.0)[:, None]
    return (g @ Wc + bc).astype(np.float32)


def sim_time_ns(edge_index, batch):
    """Cost-model (TimelineSim) predicted HW time for both launches, ns."""
    from concourse.timeline_sim import TimelineSim
    pre = _preprocess(edge_index, batch)
    key = pre["SUMNT"]
    if ("p0", key) not in _CACHE:
        _CACHE[("p0", key)] = _build_program(0, pre)
        _CACHE[("p1", key)] = _build_program(1, pre)
    t0 = TimelineSim(_CACHE[("p0", key)]).simulate()
    t1 = TimelineSim(_CACHE[("p1", key)]).simulate()
    return t0, t1
